# revision 24
# baseline (speedup 1.0000x reference)
"""Trainium2 Bass kernel for CodecLlamaCodecEmbedding (MoE-routed per-codebook MLP).

Strategy (expert-parallel): there are 8 codebooks and 8 NeuronCores. The host
sorts tokens by codebook (the MoE dispatch) and sends core k exactly the tokens
belonging to codebook k (padded to a 128-aligned capacity so the SPMD program
is static), already gathered from the embedding table and transposed to
feature-major [16, cap] bf16 layout, plus that codebook's projector weights.

Each core runs the 2-layer projector on-device in a per-tile software
pipeline L1(j+2) || gelu(j+1) || L2(j):
  layer 1:  hT = gelu(W1.T @ eT + b1) per 128-token tile (16 bf16 matmuls
            into one 4-bank PSUM tile, exact erf GELU on ScalarE with the
            bias fused, bf16 output). Per-tile granularity keeps the
            ScalarE activation sweep (~6 us/tile) comfortably ahead of
            layer 2 (~12 us/tile), so the PE is never activation-paced.
  layer 2:  out[tok, :] = hT.T @ W2 + b2 in bf16 (same 1 col/cycle PE rate
            as f32r, half the HBM/SBUF traffic; end-to-end rel err ~4e-3 vs
            the 2e-2 budget). PSUM is scheduled n-major: one 512-column
            PSUM bank accumulates all 16 contraction chunks, drains on
            VectorE (b2 add, bf16) into a per-tile staging tile that leaves
            in a single DMA.
W2 streams from HBM as bf16 pre-blocked [n][p][kc][c] so each transfer lands
16 KB contiguous per partition (16 KB DMA packets -> HBM-rate ingest, fully
resident by ~35 us), on the SP DGE queue that it owns exclusively; outputs
and b2 ride the Activation DGE queue. The host upcasts the bf16 outputs and
scatters them back to token order.
"""

import math
from contextlib import ExitStack

import numpy as np

import concourse.bacc as bacc
import concourse.tile as tile
from concourse import mybir
from concourse.bass_utils import run_bass_kernel_spmd

# Problem constants (hardcoded per the harness contract).
NUM_CODEBOOKS = 8
CODEBOOK_SIZE = 2048
D = 16        # codebook embedding dim
H = 2048      # hidden size
V = NUM_CODEBOOKS * CODEBOOK_SIZE  # embed table rows
N_CORES = 8

P = 128                  # SBUF partitions / tile edge
KC = H // P              # 16 contraction chunks for layer 2
NFREE = 512              # matmul moving-operand free dim (1 PSUM bank of fp32)
NSPLIT = H // NFREE      # 4 output column chunks

F32 = mybir.dt.float32
BF16 = mybir.dt.bfloat16


def _np_bf16():
    import ml_dtypes
    return ml_dtypes.bfloat16


TUNE = {
    "la": 8,        # bootstrap lookahead: L1 tiles emitted before L2 starts
    "ht_bufs": 10,
    "ob_bufs": 3,
    "l2_bufs": 4,
}


def _emit(ctx: ExitStack, tc: tile.TileContext, aps: dict, nt: int,
          act=mybir.ActivationFunctionType.Gelu, tune=None):
    t = dict(TUNE)
    t.update(tune or {})
    nc = tc.nc
    et_ap = aps["et"]        # [D, cap] bf16, pre-gathered transposed embeddings
    w1_ap = aps["w1"]        # [D, H]  bf16
    b1_ap = aps["b1"]        # [P, KC] f32, b1_ap[p, c] = b1[c*128 + p]
    w2_ap = aps["w2"]        # [NSPLIT*P, KC*NFREE] bf16, [n][p][kc][c] blocked
    b2_ap = aps["b2"]        # [P, H] f32, b2 replicated across partitions
    out_ap = aps["out"]      # [cap, H] bf16

    const = ctx.enter_context(tc.tile_pool(name="const", bufs=1))
    w2p = ctx.enter_context(tc.tile_pool(name="w2p", bufs=1))
    htp = ctx.enter_context(tc.tile_pool(name="htp", bufs=t["ht_bufs"]))
    op = ctx.enter_context(tc.tile_pool(name="op", bufs=t["ob_bufs"]))
    # Layer-1 PSUM: one 4-bank [128, 2048] tile holds a whole tile's hidden
    # row; with l2p's 4 single-bank accumulators that exactly fills PSUM.
    l1p = ctx.enter_context(tc.tile_pool(name="l1p", bufs=1, space="PSUM"))
    l2p = ctx.enter_context(tc.tile_pool(name="l2p", bufs=t["l2_bufs"], space="PSUM"))

    # Warm the ScalarE activation table immediately so the one-time
    # ACT_TABLE_LOAD (~1.3 us) overlaps the input DMAs instead of stalling
    # the first real GELU.
    warm_in = const.tile([1, 2], F32)
    nc.gpsimd.memset(warm_in[:], 0.0)
    warm_out = const.tile([1, 2], F32)
    nc.scalar.activation(warm_out[:], warm_in[:], act)

    # Layer-1 inputs gate the very first matmul; they ride the Activation
    # DGE queue, which is otherwise empty at the start, so their completion
    # semaphores never queue behind the 8 MB W2 stream on the SP queue.
    # et rides as ONE transfer (70 KB, lands right behind w1): a split
    # et[0]/et[1:] was measured worse — the remainder DMA's completion
    # semaphore posts several us after the data and stalled the boot tiles.
    w1_sb = const.tile([D, H], BF16)
    nc.scalar.dma_start(w1_sb[:], w1_ap[:, :])
    et_sb = const.tile([D, nt * P], BF16)
    nc.scalar.dma_start(et_sb[:], et_ap[:, :])
    b1_sb = const.tile([P, KC], F32)
    nc.scalar.dma_start(b1_sb[:], b1_ap[:, :])

    # W2 resident in SBUF as bf16, laid out n-major: block (n, kc) occupies
    # columns [(n*KC + kc)*NFREE, ...+NFREE). The host ships W2 pre-arranged
    # as [n][p][kc][c] so each n-superblock transfer lands 16 KB CONTIGUOUS
    # per partition — 16 KB DMA packets instead of the 1 KB packets a [H, H]
    # layout forces, which lifts the ingest from ~170 GB/s to HBM rate. The
    # stream owns the SP DGE queue (outputs and b2 ride the Activation
    # queue so they never delay it) and arrives in consumption order.
    w2_sb = w2p.tile([P, NSPLIT * KC * NFREE], BF16)
    b2_sb = const.tile([P, H], F32)
    nc.scalar.dma_start(b2_sb[:], b2_ap[:, :])
    SUP = KC * NFREE  # 8192 columns = 16 KB/partition per n-superblock
    for n in range(NSPLIT):
        nc.sync.dma_start(
            w2_sb[:, n * SUP:(n + 1) * SUP],
            w2_ap[n * P:(n + 1) * P, :],
        )

    def emit_l1(tt):
        ps1 = l1p.tile([P, H], F32, tag="l1")
        ht = htp.tile([P, H], BF16, tag="ht", name=f"ht_{tt}")
        eT = et_sb[:, tt * P:(tt + 1) * P]
        for hc in range(KC):
            nc.tensor.matmul(
                ps1[:, hc * P:(hc + 1) * P],
                w1_sb[:, hc * P:(hc + 1) * P],
                eT,
                start=True,
                stop=True,
            )
        for hc in range(KC):
            nc.scalar.activation(
                ht[:, hc * P:(hc + 1) * P],
                ps1[:, hc * P:(hc + 1) * P],
                act,
                bias=b1_sb[:, hc:hc + 1],
            )
        return ht

    def emit_l2(tt, ht):
        # n-major: one PSUM bank accumulates kc=0..15, then drains while the
        # next bank accumulates. Every matmul re-issues LDWEIGHTS anyway, so
        # this order costs nothing on the PE and keeps PSUM pressure at 1.
        ob = op.tile([P, H], BF16, tag="ob")
        for n in range(NSPLIT):
            ps = l2p.tile([P, NFREE], F32, tag="l2", name=f"ps2_{tt}_{n}")
            for kc in range(KC):
                nc.tensor.matmul(
                    ps[:],
                    ht[:, kc * P:(kc + 1) * P],
                    w2_sb[:, (n * KC + kc) * NFREE:(n * KC + kc + 1) * NFREE],
                    start=(kc == 0),
                    stop=(kc == KC - 1),
                )
            nc.vector.tensor_add(
                ob[:, n * NFREE:(n + 1) * NFREE], ps[:],
                b2_sb[:, n * NFREE:(n + 1) * NFREE],
            )
        nc.scalar.dma_start(out_ap[tt * P:(tt + 1) * P, :], ob[:])

    # Software pipeline with a deep bootstrap: the first `la` tiles' layer-1
    # matmuls run before any layer 2. This (a) keeps the PE busy with
    # W2-independent work until the W2 stream is fully resident (~34 us, HBM
    # rate), so layer 2 never stalls on it, and (b) keeps the PE gap-free
    # from the first matmul — the tensor engine only reaches its top p-state
    # after ~3 us of continuous execution and ANY stall drops it back to
    # 1.2 GHz, so stream continuity is worth more than an early L2 start.
    # Steady state: one L1 tile (~1.2 us) slots in behind each L2 tile
    # (~12 us) while ScalarE runs the previous tile's GELU sweep.
    la = max(1, min(t["la"], nt))
    hts = {j: emit_l1(j) for j in range(la)}
    for j in range(nt):
        emit_l2(j, hts.pop(j))
        if j + la < nt:
            hts[j + la] = emit_l1(j + la)


def build_nc(cap, act=mybir.ActivationFunctionType.Gelu, tune=None):
    assert cap % P == 0 and cap > 0
    nt = cap // P
    nc = bacc.Bacc("TRN2", target_bir_lowering=False, debug=False)
    aps = {
        "et": nc.dram_tensor("et", [D, cap], BF16, kind="ExternalInput").ap(),
        "w1": nc.dram_tensor("w1", [D, H], BF16, kind="ExternalInput").ap(),
        "b1": nc.dram_tensor("b1", [P, KC], F32, kind="ExternalInput").ap(),
        "w2": nc.dram_tensor("w2", [NSPLIT * P, KC * NFREE], BF16,
                             kind="ExternalInput").ap(),
        "b2": nc.dram_tensor("b2", [P, H], F32, kind="ExternalInput").ap(),
        "out": nc.dram_tensor("out", [cap, H], BF16, kind="ExternalOutput").ap(),
    }
    with tile.TileContext(nc) as tc:
        with ExitStack() as ctx:
            _emit(ctx, tc, aps, nt, act=act, tune=tune)
    nc.compile()
    return nc


_NC_CACHE = {}


def _get_nc(cap):
    if cap not in _NC_CACHE:
        _NC_CACHE[cap] = build_nc(cap)
    return _NC_CACHE[cap]


def _gelu_exact_np(x):
    try:
        from scipy.special import erf
    except ImportError:
        erf = np.vectorize(math.erf)
    return 0.5 * x * (1.0 + erf(x / np.sqrt(2.0).astype(x.dtype)))


def _route(ids_flat: np.ndarray):
    """Sort token positions by codebook. Returns per-codebook position lists."""
    cb = ids_flat // CODEBOOK_SIZE
    order = np.argsort(cb, kind="stable")
    counts = np.bincount(cb, minlength=NUM_CODEBOOKS)
    starts = np.concatenate([[0], np.cumsum(counts)])
    return [order[starts[k]:starts[k + 1]] for k in range(NUM_CODEBOOKS)], counts


MAX_DEV_CAP = 4096  # beyond this (a ~48-sigma skew) overflow tokens go to host


def pick_cap(counts):
    """Smallest multiple of 128 covering the max per-codebook load."""
    need = max(int(counts.max()), P)
    nt = -(-need // P)
    return min(nt * P, MAX_DEV_CAP)


def make_in_maps(ids_flat, embed_table, W1, b1, W2, b2, cap):
    positions, counts = _route(ids_flat)
    table = np.ascontiguousarray(embed_table, dtype=np.float32)
    bf16 = _np_bf16()
    in_maps = []
    for k in range(NUM_CODEBOOKS):
        pos_k = positions[k][:cap]
        idx_pad = np.zeros(cap, np.int64)  # padding points at table row 0
        idx_pad[:len(pos_k)] = ids_flat[pos_k]
        in_maps.append({
            "et": np.ascontiguousarray(table[idx_pad].T.astype(bf16)),
            "w1": np.ascontiguousarray(np.asarray(W1[k], dtype=np.float32).astype(bf16)),
            "b1": np.ascontiguousarray(np.asarray(b1[k], dtype=np.float32).reshape(KC, P).T),
            # blocked (n, p, kc, c): each n-superblock DMA lands 16 KB
            # contiguous per SBUF partition (big DMA packets, HBM-rate ingest)
            "w2": np.ascontiguousarray(
                np.asarray(W2[k], dtype=np.float32).astype(bf16)
                .reshape(KC, P, NSPLIT, NFREE).transpose(2, 1, 0, 3)
                .reshape(NSPLIT * P, KC * NFREE)
            ),
            "b2": np.ascontiguousarray(
                np.broadcast_to(np.asarray(b2[k], dtype=np.float32), (P, H))
            ),
        })
    return in_maps, positions, counts


def kernel(codec_input_ids, embed_table, W1, b1, W2, b2):
    codec_input_ids = np.asarray(codec_input_ids)
    embed_table = np.asarray(embed_table, dtype=np.float32)
    W1 = np.asarray(W1, dtype=np.float32)
    b1 = np.asarray(b1, dtype=np.float32)
    W2 = np.asarray(W2, dtype=np.float32)
    b2 = np.asarray(b2, dtype=np.float32)

    B, S = codec_input_ids.shape
    ids_flat = codec_input_ids.reshape(-1).astype(np.int64)

    _, counts = _route(ids_flat)
    cap = pick_cap(counts)
    in_maps, positions, counts = make_in_maps(
        ids_flat, embed_table, W1, b1, W2, b2, cap=cap
    )

    try:
        nc = _get_nc(cap)
        results = run_bass_kernel_spmd(nc, in_maps, list(range(N_CORES))).results
    except Exception as e:  # device/compile fault: stay correct via host math
        import sys
        print(f"kernel: device path failed ({e!r}); host fallback", file=sys.stderr)
        results = None

    out_flat = np.zeros((B * S, H), np.float32)
    for k in range(NUM_CODEBOOKS):
        pos_k = positions[k]
        n_dev = min(len(pos_k), cap) if results is not None else 0
        if n_dev:
            out_flat[pos_k[:n_dev]] = results[k]["out"][:n_dev].astype(np.float32)
        if len(pos_k) > n_dev:
            # Overflow beyond the compiled capacity (never happens for the
            # reference input distribution) or device-fault fallback:
            # compute exactly on host.
            pos_of = pos_k[n_dev:]
            e = embed_table[ids_flat[pos_of]]
            h = _gelu_exact_np(e @ W1[k] + b1[k])
            out_flat[pos_of] = h @ W2[k] + b2[k]

    return out_flat.reshape(B, S, H)


# revision 25
# speedup vs baseline: 1.1893x; 1.1893x over previous
"""Trainium2 Bass kernel for CodecLlamaCodecEmbedding (MoE-routed per-codebook MLP).

Strategy (expert-parallel): there are 8 codebooks and 8 NeuronCores. The host
sorts tokens by codebook (the MoE dispatch) and sends core k exactly the tokens
belonging to codebook k (padded to a 128-aligned capacity so the SPMD program
is static), already gathered from the embedding table and transposed to
feature-major [16, cap] bf16 layout, plus that codebook's projector weights.

Each core runs the 2-layer projector on-device in a per-tile software
pipeline L1(j+2) || gelu(j+1) || L2(j):
  layer 1:  hT = gelu(W1.T @ eT + b1) per 128-token tile (16 bf16 matmuls
            into one 4-bank PSUM tile, exact erf GELU on ScalarE with the
            bias fused, bf16 output). Per-tile granularity keeps the
            ScalarE activation sweep (~6 us/tile) comfortably ahead of
            layer 2 (~12 us/tile), so the PE is never activation-paced.
  layer 2:  out[tok, :] = hT.T @ W2 + b2 in bf16 (same 1 col/cycle PE rate
            as f32r, half the HBM/SBUF traffic; end-to-end rel err ~4e-3 vs
            the 2e-2 budget). PSUM is scheduled n-major: one 512-column
            PSUM bank accumulates all 16 contraction chunks, drains on
            VectorE (b2 add, bf16) into a per-tile staging tile that leaves
            in a single DMA.
W2 streams from HBM as bf16 pre-blocked [n][p][kc][c] so each transfer lands
16 KB contiguous per partition (16 KB DMA packets -> HBM-rate ingest, fully
resident by ~35 us), on the SP DGE queue that it owns exclusively; outputs
and b2 ride the Activation DGE queue. The host upcasts the bf16 outputs and
scatters them back to token order.
"""

import math
from contextlib import ExitStack

import numpy as np

import concourse.bacc as bacc
import concourse.tile as tile
from concourse import mybir
from concourse.bass_utils import run_bass_kernel_spmd

# Problem constants (hardcoded per the harness contract).
NUM_CODEBOOKS = 8
CODEBOOK_SIZE = 2048
D = 16        # codebook embedding dim
H = 2048      # hidden size
V = NUM_CODEBOOKS * CODEBOOK_SIZE  # embed table rows
N_CORES = 8

P = 128                  # SBUF partitions / tile edge
KC = H // P              # 16 contraction chunks for layer 2
NFREE = 512              # matmul moving-operand free dim (1 PSUM bank of fp32)
NSPLIT = H // NFREE      # 4 output column chunks

F32 = mybir.dt.float32
BF16 = mybir.dt.bfloat16


def _np_bf16():
    import ml_dtypes
    return ml_dtypes.bfloat16


TUNE = {
    "la": 8,        # bootstrap lookahead: L1 tiles emitted before L2 starts
    "ht_bufs": 10,
    "ob_bufs": 3,
    "l2_bufs": 4,
}


def _emit(ctx: ExitStack, tc: tile.TileContext, aps: dict, nt: int,
          act=mybir.ActivationFunctionType.Gelu, tune=None):
    t = dict(TUNE)
    t.update(tune or {})
    nc = tc.nc
    et_ap = aps["et"]        # [D, cap] bf16, pre-gathered transposed embeddings
    w1_ap = aps["w1"]        # [D, H]  bf16
    b1_ap = aps["b1"]        # [P, KC] f32, b1_ap[p, c] = b1[c*128 + p]
    w2_ap = aps["w2"]        # [NSPLIT*P, KC*NFREE] bf16, [n][p][kc][c] blocked
    b2_ap = aps["b2"]        # [P, H] f32, b2 replicated across partitions
    out_ap = aps["out"]      # [cap, H] bf16

    const = ctx.enter_context(tc.tile_pool(name="const", bufs=1))
    w2p = ctx.enter_context(tc.tile_pool(name="w2p", bufs=1))
    htp = ctx.enter_context(tc.tile_pool(name="htp", bufs=t["ht_bufs"]))
    op = ctx.enter_context(tc.tile_pool(name="op", bufs=t["ob_bufs"]))
    # Layer-1 PSUM: one 4-bank [128, 2048] tile holds a whole tile's hidden
    # row; with l2p's 4 single-bank accumulators that exactly fills PSUM.
    l1p = ctx.enter_context(tc.tile_pool(name="l1p", bufs=1, space="PSUM"))
    l2p = ctx.enter_context(tc.tile_pool(name="l2p", bufs=t["l2_bufs"], space="PSUM"))

    # Warm the ScalarE activation table immediately so the one-time
    # ACT_TABLE_LOAD (~1.3 us) overlaps the input DMAs instead of stalling
    # the first real GELU.
    warm_in = const.tile([1, 2], F32)
    nc.gpsimd.memset(warm_in[:], 0.0)
    warm_out = const.tile([1, 2], F32)
    nc.scalar.activation(warm_out[:], warm_in[:], act)

    # Layer-1 inputs gate the very first matmul; they ride the Activation
    # DGE queue, which is otherwise empty at the start, so their completion
    # semaphores never queue behind the 8 MB W2 stream on the SP queue.
    # et is split tile-0-slice / remainder: the tile-0 slice gates the very
    # first matmul, and a single unified et transfer was measured 54 us
    # WORSE (its completion semaphore gates every boot tile and posts late).
    w1_sb = const.tile([D, H], BF16)
    nc.scalar.dma_start(w1_sb[:], w1_ap[:, :])
    et_sb = const.tile([D, nt * P], BF16)
    nc.scalar.dma_start(et_sb[:, :P], et_ap[:, :P])
    b1_sb = const.tile([P, KC], F32)
    nc.scalar.dma_start(b1_sb[:], b1_ap[:, :])
    if nt > 1:
        nc.scalar.dma_start(et_sb[:, P:], et_ap[:, P:])

    # W2 resident in SBUF as bf16, laid out n-major: block (n, kc) occupies
    # columns [(n*KC + kc)*NFREE, ...+NFREE). The host ships W2 pre-arranged
    # as [n][p][kc][c] so each n-superblock transfer lands 16 KB CONTIGUOUS
    # per partition — 16 KB DMA packets instead of the 1 KB packets a [H, H]
    # layout forces, which lifts the ingest from ~170 GB/s to HBM rate. The
    # stream owns the SP DGE queue (outputs and b2 ride the Activation
    # queue so they never delay it) and arrives in consumption order.
    w2_sb = w2p.tile([P, NSPLIT * KC * NFREE], BF16)
    b2_sb = const.tile([P, H], F32)
    nc.scalar.dma_start(b2_sb[:], b2_ap[:, :])
    SUP = KC * NFREE  # 8192 columns = 16 KB/partition per n-superblock
    for n in range(NSPLIT):
        nc.sync.dma_start(
            w2_sb[:, n * SUP:(n + 1) * SUP],
            w2_ap[n * P:(n + 1) * P, :],
        )

    def emit_l1(tt):
        ps1 = l1p.tile([P, H], F32, tag="l1")
        ht = htp.tile([P, H], BF16, tag="ht", name=f"ht_{tt}")
        eT = et_sb[:, tt * P:(tt + 1) * P]
        for hc in range(KC):
            nc.tensor.matmul(
                ps1[:, hc * P:(hc + 1) * P],
                w1_sb[:, hc * P:(hc + 1) * P],
                eT,
                start=True,
                stop=True,
            )
        for hc in range(KC):
            nc.scalar.activation(
                ht[:, hc * P:(hc + 1) * P],
                ps1[:, hc * P:(hc + 1) * P],
                act,
                bias=b1_sb[:, hc:hc + 1],
            )
        return ht

    def emit_l2(tt, ht):
        # n-major: one PSUM bank accumulates kc=0..15, then drains while the
        # next bank accumulates. Every matmul re-issues LDWEIGHTS anyway, so
        # this order costs nothing on the PE and keeps PSUM pressure at 1.
        ob = op.tile([P, H], BF16, tag="ob")
        for n in range(NSPLIT):
            ps = l2p.tile([P, NFREE], F32, tag="l2", name=f"ps2_{tt}_{n}")
            for kc in range(KC):
                nc.tensor.matmul(
                    ps[:],
                    ht[:, kc * P:(kc + 1) * P],
                    w2_sb[:, (n * KC + kc) * NFREE:(n * KC + kc + 1) * NFREE],
                    start=(kc == 0),
                    stop=(kc == KC - 1),
                )
            nc.vector.tensor_add(
                ob[:, n * NFREE:(n + 1) * NFREE], ps[:],
                b2_sb[:, n * NFREE:(n + 1) * NFREE],
            )
        nc.scalar.dma_start(out_ap[tt * P:(tt + 1) * P, :], ob[:])

    # Software pipeline with a deep bootstrap: the first `la` tiles' layer-1
    # matmuls run before any layer 2. This (a) keeps the PE busy with
    # W2-independent work until the W2 stream is fully resident (~34 us, HBM
    # rate), so layer 2 never stalls on it, and (b) keeps the PE gap-free
    # from the first matmul — the tensor engine only reaches its top p-state
    # after ~3 us of continuous execution and ANY stall drops it back to
    # 1.2 GHz, so stream continuity is worth more than an early L2 start.
    # Steady state: one L1 tile (~1.2 us) slots in behind each L2 tile
    # (~12 us) while ScalarE runs the previous tile's GELU sweep.
    la = max(1, min(t["la"], nt))
    hts = {j: emit_l1(j) for j in range(la)}
    for j in range(nt):
        emit_l2(j, hts.pop(j))
        if j + la < nt:
            hts[j + la] = emit_l1(j + la)


def build_nc(cap, act=mybir.ActivationFunctionType.Gelu, tune=None):
    assert cap % P == 0 and cap > 0
    nt = cap // P
    nc = bacc.Bacc("TRN2", target_bir_lowering=False, debug=False)
    aps = {
        "et": nc.dram_tensor("et", [D, cap], BF16, kind="ExternalInput").ap(),
        "w1": nc.dram_tensor("w1", [D, H], BF16, kind="ExternalInput").ap(),
        "b1": nc.dram_tensor("b1", [P, KC], F32, kind="ExternalInput").ap(),
        "w2": nc.dram_tensor("w2", [NSPLIT * P, KC * NFREE], BF16,
                             kind="ExternalInput").ap(),
        "b2": nc.dram_tensor("b2", [P, H], F32, kind="ExternalInput").ap(),
        "out": nc.dram_tensor("out", [cap, H], BF16, kind="ExternalOutput").ap(),
    }
    with tile.TileContext(nc) as tc:
        with ExitStack() as ctx:
            _emit(ctx, tc, aps, nt, act=act, tune=tune)
    nc.compile()
    return nc


_NC_CACHE = {}


def _get_nc(cap):
    if cap not in _NC_CACHE:
        _NC_CACHE[cap] = build_nc(cap)
    return _NC_CACHE[cap]


def _gelu_exact_np(x):
    try:
        from scipy.special import erf
    except ImportError:
        erf = np.vectorize(math.erf)
    return 0.5 * x * (1.0 + erf(x / np.sqrt(2.0).astype(x.dtype)))


def _route(ids_flat: np.ndarray):
    """Sort token positions by codebook. Returns per-codebook position lists."""
    cb = ids_flat // CODEBOOK_SIZE
    order = np.argsort(cb, kind="stable")
    counts = np.bincount(cb, minlength=NUM_CODEBOOKS)
    starts = np.concatenate([[0], np.cumsum(counts)])
    return [order[starts[k]:starts[k + 1]] for k in range(NUM_CODEBOOKS)], counts


MAX_DEV_CAP = 4096  # beyond this (a ~48-sigma skew) overflow tokens go to host


def pick_cap(counts):
    """Smallest multiple of 128 covering the max per-codebook load."""
    need = max(int(counts.max()), P)
    nt = -(-need // P)
    return min(nt * P, MAX_DEV_CAP)


def make_in_maps(ids_flat, embed_table, W1, b1, W2, b2, cap):
    positions, counts = _route(ids_flat)
    table = np.ascontiguousarray(embed_table, dtype=np.float32)
    bf16 = _np_bf16()
    in_maps = []
    for k in range(NUM_CODEBOOKS):
        pos_k = positions[k][:cap]
        idx_pad = np.zeros(cap, np.int64)  # padding points at table row 0
        idx_pad[:len(pos_k)] = ids_flat[pos_k]
        in_maps.append({
            "et": np.ascontiguousarray(table[idx_pad].T.astype(bf16)),
            "w1": np.ascontiguousarray(np.asarray(W1[k], dtype=np.float32).astype(bf16)),
            "b1": np.ascontiguousarray(np.asarray(b1[k], dtype=np.float32).reshape(KC, P).T),
            # blocked (n, p, kc, c): each n-superblock DMA lands 16 KB
            # contiguous per SBUF partition (big DMA packets, HBM-rate ingest)
            "w2": np.ascontiguousarray(
                np.asarray(W2[k], dtype=np.float32).astype(bf16)
                .reshape(KC, P, NSPLIT, NFREE).transpose(2, 1, 0, 3)
                .reshape(NSPLIT * P, KC * NFREE)
            ),
            "b2": np.ascontiguousarray(
                np.broadcast_to(np.asarray(b2[k], dtype=np.float32), (P, H))
            ),
        })
    return in_maps, positions, counts


def kernel(codec_input_ids, embed_table, W1, b1, W2, b2):
    codec_input_ids = np.asarray(codec_input_ids)
    embed_table = np.asarray(embed_table, dtype=np.float32)
    W1 = np.asarray(W1, dtype=np.float32)
    b1 = np.asarray(b1, dtype=np.float32)
    W2 = np.asarray(W2, dtype=np.float32)
    b2 = np.asarray(b2, dtype=np.float32)

    B, S = codec_input_ids.shape
    ids_flat = codec_input_ids.reshape(-1).astype(np.int64)

    _, counts = _route(ids_flat)
    cap = pick_cap(counts)
    in_maps, positions, counts = make_in_maps(
        ids_flat, embed_table, W1, b1, W2, b2, cap=cap
    )

    try:
        nc = _get_nc(cap)
        results = run_bass_kernel_spmd(nc, in_maps, list(range(N_CORES))).results
    except Exception as e:  # device/compile fault: stay correct via host math
        import sys
        print(f"kernel: device path failed ({e!r}); host fallback", file=sys.stderr)
        results = None

    out_flat = np.zeros((B * S, H), np.float32)
    for k in range(NUM_CODEBOOKS):
        pos_k = positions[k]
        n_dev = min(len(pos_k), cap) if results is not None else 0
        if n_dev:
            out_flat[pos_k[:n_dev]] = results[k]["out"][:n_dev].astype(np.float32)
        if len(pos_k) > n_dev:
            # Overflow beyond the compiled capacity (never happens for the
            # reference input distribution) or device-fault fallback:
            # compute exactly on host.
            pos_of = pos_k[n_dev:]
            e = embed_table[ids_flat[pos_of]]
            h = _gelu_exact_np(e @ W1[k] + b1[k])
            out_flat[pos_of] = h @ W2[k] + b2[k]

    return out_flat.reshape(B, S, H)


# revision 27
# speedup vs baseline: 1.1899x; 1.0005x over previous
"""Trainium2 Bass kernel for CodecLlamaCodecEmbedding (MoE-routed per-codebook MLP).

Strategy (expert-parallel): there are 8 codebooks and 8 NeuronCores. The host
sorts tokens by codebook (the MoE dispatch) and sends core k exactly the tokens
belonging to codebook k (padded to a 128-aligned capacity so the SPMD program
is static), already gathered from the embedding table and transposed to
feature-major [16, cap] bf16 layout, plus that codebook's projector weights.

Each core runs the 2-layer projector on-device in a per-tile software
pipeline L1(j+2) || gelu(j+1) || L2(j):
  layer 1:  hT = gelu(W1.T @ eT + b1) per 128-token tile (16 bf16 matmuls
            into one 4-bank PSUM tile, exact erf GELU on ScalarE with the
            bias fused, bf16 output). Per-tile granularity keeps the
            ScalarE activation sweep (~6 us/tile) comfortably ahead of
            layer 2 (~12 us/tile), so the PE is never activation-paced.
  layer 2:  out[tok, :] = hT.T @ W2 + b2 in bf16 (same 1 col/cycle PE rate
            as f32r, half the HBM/SBUF traffic; end-to-end rel err ~4e-3 vs
            the 2e-2 budget). PSUM is scheduled n-major: one 512-column
            PSUM bank accumulates all 16 contraction chunks, drains on
            VectorE (b2 add, bf16) into a per-tile staging tile that leaves
            in a single DMA.
W2 streams from HBM as bf16 pre-blocked [n][p][kc][c] so each transfer lands
16 KB contiguous per partition (16 KB DMA packets -> HBM-rate ingest, fully
resident by ~35 us), on the SP DGE queue that it owns exclusively; outputs
and b2 ride the Activation DGE queue. The host upcasts the bf16 outputs and
scatters them back to token order.
"""

import math
from contextlib import ExitStack

import numpy as np

import concourse.bacc as bacc
import concourse.tile as tile
from concourse import mybir
from concourse.bass_utils import run_bass_kernel_spmd

# Problem constants (hardcoded per the harness contract).
NUM_CODEBOOKS = 8
CODEBOOK_SIZE = 2048
D = 16        # codebook embedding dim
H = 2048      # hidden size
V = NUM_CODEBOOKS * CODEBOOK_SIZE  # embed table rows
N_CORES = 8

P = 128                  # SBUF partitions / tile edge
KC = H // P              # 16 contraction chunks for layer 2
NFREE = 512              # matmul moving-operand free dim (1 PSUM bank of fp32)
NSPLIT = H // NFREE      # 4 output column chunks

F32 = mybir.dt.float32
BF16 = mybir.dt.bfloat16


def _np_bf16():
    import ml_dtypes
    return ml_dtypes.bfloat16


TUNE = {
    "la": 8,        # bootstrap lookahead: L1 tiles emitted before L2 starts
    "ht_bufs": 10,
    "ob_bufs": 3,
    "l2_bufs": 4,
}


def _emit(ctx: ExitStack, tc: tile.TileContext, aps: dict, nt: int,
          act=mybir.ActivationFunctionType.Gelu, tune=None):
    t = dict(TUNE)
    t.update(tune or {})
    nc = tc.nc
    et_ap = aps["et"]        # [D, cap] bf16, pre-gathered transposed embeddings
    w1_ap = aps["w1"]        # [D, H]  bf16
    b1_ap = aps["b1"]        # [P, KC] f32, b1_ap[p, c] = b1[c*128 + p]
    w2_ap = aps["w2"]        # [NSPLIT*P, KC*NFREE] bf16, [n][p][kc][c] blocked
    b2_ap = aps["b2"]        # [P, H] f32, b2 replicated across partitions
    out_ap = aps["out"]      # [cap, H] bf16

    const = ctx.enter_context(tc.tile_pool(name="const", bufs=1))
    w2p = ctx.enter_context(tc.tile_pool(name="w2p", bufs=1))
    htp = ctx.enter_context(tc.tile_pool(name="htp", bufs=t["ht_bufs"]))
    op = ctx.enter_context(tc.tile_pool(name="op", bufs=t["ob_bufs"]))
    # Layer-1 PSUM: one 4-bank [128, 2048] tile holds a whole tile's hidden
    # row; with l2p's 4 single-bank accumulators that exactly fills PSUM.
    l1p = ctx.enter_context(tc.tile_pool(name="l1p", bufs=1, space="PSUM"))
    l2p = ctx.enter_context(tc.tile_pool(name="l2p", bufs=t["l2_bufs"], space="PSUM"))

    # Warm the ScalarE activation table immediately so the one-time
    # ACT_TABLE_LOAD (~1.3 us) overlaps the input DMAs instead of stalling
    # the first real GELU.
    warm_in = const.tile([1, 2], F32)
    nc.gpsimd.memset(warm_in[:], 0.0)
    warm_out = const.tile([1, 2], F32)
    nc.scalar.activation(warm_out[:], warm_in[:], act)

    # Layer-1 inputs gate the very first matmul; they ride the Activation
    # DGE queue, which is otherwise empty at the start, so their completion
    # semaphores never queue behind the 8 MB W2 stream on the SP queue.
    # et is split tile-0-slice / remainder: the tile-0 slice gates the very
    # first matmul, and a single unified et transfer was measured 54 us
    # WORSE (its completion semaphore gates every boot tile and posts late).
    w1_sb = const.tile([D, H], BF16)
    nc.scalar.dma_start(w1_sb[:], w1_ap[:, :])
    et_sb = const.tile([D, nt * P], BF16)
    nc.scalar.dma_start(et_sb[:, :P], et_ap[:, :P])
    b1_sb = const.tile([P, KC], F32)
    nc.scalar.dma_start(b1_sb[:], b1_ap[:, :])
    if nt > 1:
        nc.scalar.dma_start(et_sb[:, P:], et_ap[:, P:])

    # W2 resident in SBUF as bf16, laid out n-major: block (n, kc) occupies
    # columns [(n*KC + kc)*NFREE, ...+NFREE). The host ships W2 pre-arranged
    # as [n][p][kc][c] so each n-superblock transfer lands 16 KB CONTIGUOUS
    # per partition — 16 KB DMA packets instead of the 1 KB packets a [H, H]
    # layout forces, which lifts the ingest from ~170 GB/s to HBM rate. The
    # stream owns the SP DGE queue (outputs and b2 ride the Activation
    # queue so they never delay it) and arrives in consumption order.
    w2_sb = w2p.tile([P, NSPLIT * KC * NFREE], BF16)
    b2_sb = const.tile([P, H], F32)
    nc.scalar.dma_start(b2_sb[:], b2_ap[:, :])
    SUP = KC * NFREE  # 8192 columns = 16 KB/partition per n-superblock
    for n in range(NSPLIT):
        nc.sync.dma_start(
            w2_sb[:, n * SUP:(n + 1) * SUP],
            w2_ap[n * P:(n + 1) * P, :],
        )

    def emit_l1(tt):
        ps1 = l1p.tile([P, H], F32, tag="l1")
        ht = htp.tile([P, H], BF16, tag="ht", name=f"ht_{tt}")
        eT = et_sb[:, tt * P:(tt + 1) * P]
        for hc in range(KC):
            nc.tensor.matmul(
                ps1[:, hc * P:(hc + 1) * P],
                w1_sb[:, hc * P:(hc + 1) * P],
                eT,
                start=True,
                stop=True,
            )
        for hc in range(KC):
            nc.scalar.activation(
                ht[:, hc * P:(hc + 1) * P],
                ps1[:, hc * P:(hc + 1) * P],
                act,
                bias=b1_sb[:, hc:hc + 1],
            )
        return ht

    def emit_l2(tt, ht, split_out=False):
        # n-major: one PSUM bank accumulates kc=0..15, then drains while the
        # next bank accumulates. Every matmul re-issues LDWEIGHTS anyway, so
        # this order costs nothing on the PE and keeps PSUM pressure at 1.
        # split_out (last tile only): DMA each n-block as it drains, so most
        # of the output transfer overlaps the remaining matmuls instead of
        # sitting serially in the kernel tail.
        ob = op.tile([P, H], BF16, tag="ob")
        for n in range(NSPLIT):
            ps = l2p.tile([P, NFREE], F32, tag="l2", name=f"ps2_{tt}_{n}")
            for kc in range(KC):
                nc.tensor.matmul(
                    ps[:],
                    ht[:, kc * P:(kc + 1) * P],
                    w2_sb[:, (n * KC + kc) * NFREE:(n * KC + kc + 1) * NFREE],
                    start=(kc == 0),
                    stop=(kc == KC - 1),
                )
            nc.vector.tensor_add(
                ob[:, n * NFREE:(n + 1) * NFREE], ps[:],
                b2_sb[:, n * NFREE:(n + 1) * NFREE],
            )
            if split_out:
                nc.scalar.dma_start(
                    out_ap[tt * P:(tt + 1) * P, n * NFREE:(n + 1) * NFREE],
                    ob[:, n * NFREE:(n + 1) * NFREE],
                )
        if not split_out:
            nc.scalar.dma_start(out_ap[tt * P:(tt + 1) * P, :], ob[:])

    # Software pipeline with a deep bootstrap: the first `la` tiles' layer-1
    # matmuls run before any layer 2. This (a) keeps the PE busy with
    # W2-independent work until the W2 stream is fully resident (~34 us, HBM
    # rate), so layer 2 never stalls on it, and (b) keeps the PE gap-free
    # from the first matmul — the tensor engine only reaches its top p-state
    # after ~3 us of continuous execution and ANY stall drops it back to
    # 1.2 GHz, so stream continuity is worth more than an early L2 start.
    # Steady state: one L1 tile (~1.2 us) slots in behind each L2 tile
    # (~12 us) while ScalarE runs the previous tile's GELU sweep.
    la = max(1, min(t["la"], nt))
    hts = {j: emit_l1(j) for j in range(la)}
    for j in range(nt):
        emit_l2(j, hts.pop(j), split_out=(j == nt - 1))
        if j + la < nt:
            hts[j + la] = emit_l1(j + la)


def build_nc(cap, act=mybir.ActivationFunctionType.Gelu, tune=None):
    assert cap % P == 0 and cap > 0
    nt = cap // P
    nc = bacc.Bacc("TRN2", target_bir_lowering=False, debug=False)
    aps = {
        "et": nc.dram_tensor("et", [D, cap], BF16, kind="ExternalInput").ap(),
        "w1": nc.dram_tensor("w1", [D, H], BF16, kind="ExternalInput").ap(),
        "b1": nc.dram_tensor("b1", [P, KC], F32, kind="ExternalInput").ap(),
        "w2": nc.dram_tensor("w2", [NSPLIT * P, KC * NFREE], BF16,
                             kind="ExternalInput").ap(),
        "b2": nc.dram_tensor("b2", [P, H], F32, kind="ExternalInput").ap(),
        "out": nc.dram_tensor("out", [cap, H], BF16, kind="ExternalOutput").ap(),
    }
    with tile.TileContext(nc) as tc:
        with ExitStack() as ctx:
            _emit(ctx, tc, aps, nt, act=act, tune=tune)
    nc.compile()
    return nc


_NC_CACHE = {}


def _get_nc(cap):
    if cap not in _NC_CACHE:
        _NC_CACHE[cap] = build_nc(cap)
    return _NC_CACHE[cap]


def _gelu_exact_np(x):
    try:
        from scipy.special import erf
    except ImportError:
        erf = np.vectorize(math.erf)
    return 0.5 * x * (1.0 + erf(x / np.sqrt(2.0).astype(x.dtype)))


def _route(ids_flat: np.ndarray):
    """Sort token positions by codebook. Returns per-codebook position lists."""
    cb = ids_flat // CODEBOOK_SIZE
    order = np.argsort(cb, kind="stable")
    counts = np.bincount(cb, minlength=NUM_CODEBOOKS)
    starts = np.concatenate([[0], np.cumsum(counts)])
    return [order[starts[k]:starts[k + 1]] for k in range(NUM_CODEBOOKS)], counts


MAX_DEV_CAP = 4096  # beyond this (a ~48-sigma skew) overflow tokens go to host


def pick_cap(counts):
    """Smallest multiple of 128 covering the max per-codebook load."""
    need = max(int(counts.max()), P)
    nt = -(-need // P)
    return min(nt * P, MAX_DEV_CAP)


def make_in_maps(ids_flat, embed_table, W1, b1, W2, b2, cap):
    positions, counts = _route(ids_flat)
    table = np.ascontiguousarray(embed_table, dtype=np.float32)
    bf16 = _np_bf16()
    in_maps = []
    for k in range(NUM_CODEBOOKS):
        pos_k = positions[k][:cap]
        idx_pad = np.zeros(cap, np.int64)  # padding points at table row 0
        idx_pad[:len(pos_k)] = ids_flat[pos_k]
        in_maps.append({
            "et": np.ascontiguousarray(table[idx_pad].T.astype(bf16)),
            "w1": np.ascontiguousarray(np.asarray(W1[k], dtype=np.float32).astype(bf16)),
            "b1": np.ascontiguousarray(np.asarray(b1[k], dtype=np.float32).reshape(KC, P).T),
            # blocked (n, p, kc, c): each n-superblock DMA lands 16 KB
            # contiguous per SBUF partition (big DMA packets, HBM-rate ingest)
            "w2": np.ascontiguousarray(
                np.asarray(W2[k], dtype=np.float32).astype(bf16)
                .reshape(KC, P, NSPLIT, NFREE).transpose(2, 1, 0, 3)
                .reshape(NSPLIT * P, KC * NFREE)
            ),
            "b2": np.ascontiguousarray(
                np.broadcast_to(np.asarray(b2[k], dtype=np.float32), (P, H))
            ),
        })
    return in_maps, positions, counts


def kernel(codec_input_ids, embed_table, W1, b1, W2, b2):
    codec_input_ids = np.asarray(codec_input_ids)
    embed_table = np.asarray(embed_table, dtype=np.float32)
    W1 = np.asarray(W1, dtype=np.float32)
    b1 = np.asarray(b1, dtype=np.float32)
    W2 = np.asarray(W2, dtype=np.float32)
    b2 = np.asarray(b2, dtype=np.float32)

    B, S = codec_input_ids.shape
    ids_flat = codec_input_ids.reshape(-1).astype(np.int64)

    _, counts = _route(ids_flat)
    cap = pick_cap(counts)
    in_maps, positions, counts = make_in_maps(
        ids_flat, embed_table, W1, b1, W2, b2, cap=cap
    )

    try:
        nc = _get_nc(cap)
        results = run_bass_kernel_spmd(nc, in_maps, list(range(N_CORES))).results
    except Exception as e:  # device/compile fault: stay correct via host math
        import sys
        print(f"kernel: device path failed ({e!r}); host fallback", file=sys.stderr)
        results = None

    out_flat = np.zeros((B * S, H), np.float32)
    for k in range(NUM_CODEBOOKS):
        pos_k = positions[k]
        n_dev = min(len(pos_k), cap) if results is not None else 0
        if n_dev:
            out_flat[pos_k[:n_dev]] = results[k]["out"][:n_dev].astype(np.float32)
        if len(pos_k) > n_dev:
            # Overflow beyond the compiled capacity (never happens for the
            # reference input distribution) or device-fault fallback:
            # compute exactly on host.
            pos_of = pos_k[n_dev:]
            e = embed_table[ids_flat[pos_of]]
            h = _gelu_exact_np(e @ W1[k] + b1[k])
            out_flat[pos_of] = h @ W2[k] + b2[k]

    return out_flat.reshape(B, S, H)
